# revision 29
# baseline (speedup 1.0000x reference)
"""Causal depthwise conv1d with learnable hidden-state prefix, on 8 TRN2 cores.

Reference computation (per batch b, channel d):
    xp = concat([init_state[d, :3], x[b, d, :]])          # [L+3] = [4099]
    out[b, d, t] = bias[d] + sum_{j=0..3} w[d, j] * xp[t+j]   for t in [0, 4099)
    (xp index beyond 4098 contributes 0)

Sharding: channel dim D=4096 split 8 ways (512 channels/core), zero
communication. Each core processes rows (b, d_local) = 4*512 = 2048 rows of
length 4096 -> 16 SBUF tiles of [128 rows, full row].

The output is stored to DRAM in bf16 (a single final rounding, ~2e-3 relative
error vs the 2e-2 gate; all accumulation stays fp32) and upcast to fp32 on the
host. That halves output DMA traffic, dropping the per-core DMA floor from
~187us (fp32 in+out) to ~140us (fp32 in + bf16 out).

At a 140us floor the old two-engine compute split (PE+DVE at ~153us each)
becomes the bottleneck, so the output columns are split two ways, both under
the DMA roofline (per-tile DMA budget is 8.75us: 5.83 in + 2.92 out):
  - PE "z-mode" chunks, cols [0, 1632): ACT pre-writes tap0 + bias into the
    PSUM bank, then only 3 diagonal-weight fp32 matmuls accumulate on top
    (start=False), ACT evacuates -> bf16. 3 matmuls/col instead of 4 keeps
    PE at ~8.2us/tile, inside the budget.
  - DVE region, cols [1632, 4099): ACT writes tap0+bias to an fp32 scratch,
    DVE chains taps 1-2 into scratch (scalar_tensor_tensor), tap 3 reads
    scratch and writes bf16 out directly (single rounding). ~8.2us/tile.
    The 3 zero-padded in_t cols let the last taps run off the end of x.
(Pool/GPSIMD only issues the SWDGE out-DMAs: walrus rejects
scalar_tensor_tensor on Pool, and with z-mode it isn't needed.)
"""

import numpy as np

B, D, L = 4, 4096, 4096
KTAPS = 4
K = KTAPS - 1          # 3: state length
LOUT = L + K           # 4099
NCORES = 8
DSH = D // NCORES      # 512 channels per core
ROWS = B * DSH         # 2048 rows per core
P = 128                # SBUF partitions
NTILES = ROWS // P     # 16
G = DSH // P           # 4 channel groups per core

_CACHE = {}

MMCOLS = 512           # one PSUM bank of fp32 per matmul chunk
ZCHUNKS = (512, 256)        # PSUM-preload chunks: PE covers cols [0, 768)
ABCOLS = 2231               # AB-split region (ACT 2x + DVE 2x + Pool add)
DCOLS = 550                 # DVE-finished scratch cols; Pool finishes rest


def _build_program(zchunks=ZCHUNKS, xchunks=0, abcols=ABCOLS, dcols=DCOLS,
                   use_pool=True,
                   in_bufs=5, out_bufs=5, sc_bufs=3, tmp_bufs=2, preissue=1,
                   split_out=(), out_eng="pool"):
    import concourse.bacc as bacc
    import concourse.mybir as mybir
    from concourse.tile import TileContext

    f32 = mybir.dt.float32
    bf16 = mybir.dt.bfloat16
    nc = bacc.Bacc("TRN2", target_bir_lowering=False, debug=False)

    xs = nc.dram_tensor("xs", [ROWS, L], f32, kind="ExternalInput").ap()
    # single packed param tensor -> single DMA -> single sync wait downstream.
    # layout per partition p: cols [g*4+j]=w[g*128+p, j] for g<4,j<4 (0..16),
    # col 16+g = bias[g*128+p], col 20+g*3+k = init_state[g*128+p, k]
    prm_d = nc.dram_tensor("prm", [P, 32], f32, kind="ExternalInput").ap()
    eye_d = nc.dram_tensor("eye", [P, P], f32, kind="ExternalInput").ap()
    out_d = nc.dram_tensor("out", [ROWS, LOUT], bf16, kind="ExternalOutput").ap()

    chunks = [(MMCOLS, False)] * xchunks + [(n, True) for n in zchunks]
    ncols = sum(n for n, _ in chunks)     # PE region [0, ncols)
    if not use_pool:
        abcols = 0
        dcols = LOUT - ncols
    a0 = ncols                            # AB-split region [a0, d0)
    d0 = a0 + abcols                      # scratch region [d0, LOUT)
    q0 = d0 + dcols                       # Pool-finished cols [q0, LOUT)
    scw = LOUT - d0                       # scratch width
    mcols = LOUT - q0

    # in_t layout: col 0 pad (16B align), state [1:4), x [4:4100),
    # zero tail [4100:4103) so the last taps can run off the end of x.
    XW = 1 + K + L + 4     # 4104 (16B-aligned row)

    with TileContext(nc) as tc:
        with (
            tc.tile_pool(name="consts", bufs=1) as cpool,
            tc.tile_pool(name="xin", bufs=in_bufs) as in_pool,
            tc.tile_pool(name="yout", bufs=out_bufs) as out_pool,
            tc.tile_pool(name="scr", bufs=sc_bufs) as sc_pool,
            tc.tile_pool(name="ab", bufs=tmp_bufs) as ab_pool,
            tc.tile_pool(name="tmp", bufs=tmp_bufs) as tmp_pool,
            tc.tile_pool(name="psum", bufs=8, space="PSUM") as ps_pool,
        ):
            # First in-DMAs go FIRST on the SP ring: the first big transfer
            # starts as early as the pipe allows and hides the small prm/eye
            # transfers' DGE latency behind it.
            pre = {}
            for t in range(preissue):
                in_t = in_pool.tile([P, XW], f32, name="in_t", tag="in_t")
                nc.sync.dma_start(out=in_t[:, 1 + K:1 + K + L],
                                  in_=xs[t * P:(t + 1) * P, :])
                pre[t] = in_t

            prm = cpool.tile([P, 32], f32)
            nc.sync.dma_start(out=prm, in_=prm_d)
            w_sb = prm[:, 0:G * KTAPS]
            b_sb = prm[:, 16:16 + G]
            s_sb = prm[:, 20:20 + G * K]

            # per-(group, tap) diagonal weight matrices for the PE path
            eye = cpool.tile([P, P], f32)
            nc.sync.dma_start(out=eye, in_=eye_d)
            dg = {}
            for g in range(G):
                for j in range(KTAPS):
                    d = cpool.tile([P, P], f32, tag=f"diag{g}_{j}")
                    nc.vector.tensor_scalar_mul(
                        out=d, in0=eye,
                        scalar1=w_sb[:, g * KTAPS + j:g * KTAPS + j + 1])
                    dg[(g, j)] = d

            def stt(eng, out_t, in0, scal, in1):
                """out = in0*scal + in1 (fused MAC on eng)"""
                eng.scalar_tensor_tensor(
                    out=out_t, in0=in0, scalar=scal, in1=in1,
                    op0=mybir.AluOpType.mult, op1=mybir.AluOpType.add)

            for t in range(NTILES):
                g = t % G  # channel group (tile order: batch-major)
                rows = slice(t * P, (t + 1) * P)
                wj = [w_sb[:, g * KTAPS + j:g * KTAPS + j + 1]
                      for j in range(KTAPS)]

                if t in pre:
                    in_t = pre[t]
                else:
                    in_t = in_pool.tile([P, XW], f32, name="in_t", tag="in_t")
                    nc.sync.dma_start(out=in_t[:, 1 + K:1 + K + L],
                                      in_=xs[rows, :])
                nc.scalar.copy(in_t[:, 1:1 + K], s_sb[:, g * K:(g + 1) * K])
                nc.vector.memset(in_t[:, 1 + K + L:1 + K + L + K], 0.0)

                out_t = out_pool.tile([P, LOUT], bf16)

                # PE region: out[:, 0:ncols) accumulated in PSUM, ACT
                # evacuates + bias -> bf16. z-mode chunks pre-write tap0+bias
                # on ACT into the PSUM bank so only 3 matmuls accumulate on
                # top; x-mode chunks do all 4 taps as matmuls. All preloads
                # are issued before the matmul groups so the in-order ACT
                # ring stays ahead of the PE instead of serializing chunks.
                pss = []
                base = 1
                for n, zmode in chunks:
                    ps = ps_pool.tile([P, MMCOLS], f32)
                    pss.append(ps)
                    if zmode:
                        nc.scalar.activation(
                            ps[:, :n], in_t[:, base:base + n],
                            mybir.ActivationFunctionType.Identity,
                            bias=b_sb[:, g:g + 1], scale=wj[0])
                    base += n

                # AB-split region [a0, d0): the 4 taps split into two
                # independent fp32 partial sums A = w0*x0+b + w1*x1 and
                # B = w2*x2 + w3*x3 (ACT writes each base, DVE one fused MAC
                # each), and the otherwise-idle Pool adds A+B -> bf16 out
                # (single rounding). Spreads one column across 3 engines.
                if abcols:
                    sa = ab_pool.tile([P, abcols], f32, tag="sa")
                    sb = ab_pool.tile([P, abcols], f32, tag="sb")
                    nc.scalar.activation(
                        sa, in_t[:, 1 + a0:1 + a0 + abcols],
                        mybir.ActivationFunctionType.Identity,
                        bias=b_sb[:, g:g + 1], scale=wj[0])
                    nc.scalar.mul(
                        sb, in_t[:, 3 + a0:3 + a0 + abcols], wj[2])
                    stt(nc.vector, sa,
                        in_t[:, 2 + a0:2 + a0 + abcols], wj[1], sa)
                    stt(nc.vector, sb,
                        in_t[:, 4 + a0:4 + a0 + abcols], wj[3], sb)
                    nc.gpsimd.tensor_tensor(
                        out=out_t[:, a0:d0], in0=sa, in1=sb,
                        op=mybir.AluOpType.add)

                # Scratch-region tap0 on ACT.
                sc = sc_pool.tile([P, scw], f32)
                nc.scalar.activation(
                    sc, in_t[:, 1 + d0:1 + LOUT],
                    mybir.ActivationFunctionType.Identity,
                    bias=b_sb[:, g:g + 1], scale=wj[0])

                base = 1
                for ps, (n, zmode) in zip(pss, chunks):
                    for j in range(KTAPS):
                        if zmode and j == 0:
                            continue
                        nc.tensor.matmul(
                            ps[:, :n], dg[(g, j)],
                            in_t[:, base + j:base + j + n],
                            start=(j == 0), stop=(j == KTAPS - 1),
                            skip_group_check=zmode)
                    nc.scalar.activation(
                        out_t[:, base - 1:base - 1 + n], ps[:, :n],
                        mybir.ActivationFunctionType.Identity,
                        bias=(0.0 if zmode else b_sb[:, g:g + 1]),
                        scale=1.0)
                    base += n

                # Scratch region: DVE chains taps 1-2 into scratch over the
                # full width. Tap 3 is split: DVE finishes cols [d0, q0)
                # with a fused MAC writing bf16 out (single rounding); Pool
                # finishes [q0, LOUT) as tensor_scalar (tmp = x*w3) +
                # tensor_tensor add (out = tmp + scratch) since walrus has
                # no Pool scalar_tensor_tensor.
                for j in (1, 2):
                    stt(nc.vector, sc,
                        in_t[:, 1 + d0 + j:1 + LOUT + j], wj[j], sc)
                stt(nc.vector, out_t[:, d0:q0],
                    in_t[:, 1 + d0 + 3:1 + q0 + 3], wj[3],
                    sc[:, 0:q0 - d0])
                if use_pool:
                    tmp = tmp_pool.tile([P, mcols], f32)
                    nc.gpsimd.tensor_scalar(
                        out=tmp, in0=in_t[:, 1 + q0 + 3:1 + LOUT + 3],
                        scalar1=wj[3], scalar2=None,
                        op0=mybir.AluOpType.mult)
                    nc.gpsimd.tensor_tensor(
                        out=out_t[:, q0:], in0=tmp,
                        in1=sc[:, q0 - d0:], op=mybir.AluOpType.add)

                # SWDGE path: waits stall only the idle Pool sequencer;
                # the in-DMA HWDGE ring stays wait-free.
                oe = {"pool": nc.gpsimd, "act": nc.scalar, "sp": nc.sync,
                      "dve": nc.vector}[out_eng]
                if t in split_out:
                    # PE+DVE piece leaves as soon as their writes land; only
                    # the Pool-finished cols trail (shorter drain).
                    oe.dma_start(out=out_d[rows, :q0], in_=out_t[:, :q0])
                    oe.dma_start(out=out_d[rows, q0:], in_=out_t[:, q0:])
                else:
                    oe.dma_start(out=out_d[rows, :], in_=out_t)

    nc.compile()
    return nc


def kernel(x, weight, bias, init_state):
    from concourse.bass_utils import run_bass_kernel_spmd

    assert x.shape == (B, D, L) and x.dtype == np.float32
    wl = np.ascontiguousarray(weight[:, 0, :], dtype=np.float32)      # [D, 4]
    bias = np.ascontiguousarray(bias, dtype=np.float32)               # [D]
    st = np.ascontiguousarray(init_state, dtype=np.float32)           # [D, 3]

    if "nc" not in _CACHE:
        _CACHE["nc"] = _build_program()
    nc = _CACHE["nc"]

    in_maps = []
    for c in range(NCORES):
        lo, hi = c * DSH, (c + 1) * DSH
        xs = np.ascontiguousarray(x[:, lo:hi, :]).reshape(ROWS, L)
        wc = wl[lo:hi]                                                # [512, 4]
        prm = np.zeros((P, 32), np.float32)
        prm[:, 0:G * KTAPS] = (
            wc.reshape(G, P, KTAPS).transpose(1, 0, 2).reshape(P, G * KTAPS))
        prm[:, 16:16 + G] = bias[lo:hi].reshape(G, P).T
        prm[:, 20:20 + G * K] = (
            st[lo:hi].reshape(G, P, K).transpose(1, 0, 2).reshape(P, G * K))
        in_maps.append({"xs": xs, "prm": prm,
                        "eye": np.eye(P, dtype=np.float32)})

    res = run_bass_kernel_spmd(nc, in_maps, core_ids=list(range(NCORES)))
    shards = [r["out"].astype(np.float32).reshape(B, DSH, LOUT)
              for r in res.results]
    return np.ascontiguousarray(np.concatenate(shards, axis=1))


# revision 31
# speedup vs baseline: 1.0855x; 1.0855x over previous
"""Causal depthwise conv1d with learnable hidden-state prefix, on 8 TRN2 cores.

Reference computation (per batch b, channel d):
    xp = concat([init_state[d, :3], x[b, d, :]])          # [L+3] = [4099]
    out[b, d, t] = bias[d] + sum_{j=0..3} w[d, j] * xp[t+j]   for t in [0, 4099)
    (xp index beyond 4098 contributes 0)

Sharding: channel dim D=4096 split 8 ways (512 channels/core), zero
communication. Each core processes rows (b, d_local) = 4*512 = 2048 rows of
length 4096 -> 16 SBUF tiles of [128 rows, full row].

The output is stored to DRAM in bf16 (a single final rounding, ~2e-3 relative
error vs the 2e-2 gate; all accumulation stays fp32) and upcast to fp32 on the
host. That halves output DMA traffic, dropping the per-core DMA floor from
~187us (fp32 in+out) to ~140us (fp32 in + bf16 out).

At a 140us floor the old two-engine compute split (PE+DVE at ~153us each)
becomes the bottleneck, so the output columns are split two ways, both under
the DMA roofline (per-tile DMA budget is 8.75us: 5.83 in + 2.92 out):
  - PE "z-mode" chunks, cols [0, 1632): ACT pre-writes tap0 + bias into the
    PSUM bank, then only 3 diagonal-weight fp32 matmuls accumulate on top
    (start=False), ACT evacuates -> bf16. 3 matmuls/col instead of 4 keeps
    PE at ~8.2us/tile, inside the budget.
  - DVE region, cols [1632, 4099): ACT writes tap0+bias to an fp32 scratch,
    DVE chains taps 1-2 into scratch (scalar_tensor_tensor), tap 3 reads
    scratch and writes bf16 out directly (single rounding). ~8.2us/tile.
    The 3 zero-padded in_t cols let the last taps run off the end of x.
(Pool/GPSIMD only issues the SWDGE out-DMAs: walrus rejects
scalar_tensor_tensor on Pool, and with z-mode it isn't needed.)
"""

import numpy as np

B, D, L = 4, 4096, 4096
KTAPS = 4
K = KTAPS - 1          # 3: state length
LOUT = L + K           # 4099
NCORES = 8
DSH = D // NCORES      # 512 channels per core
ROWS = B * DSH         # 2048 rows per core
P = 128                # SBUF partitions
NTILES = ROWS // P     # 16
G = DSH // P           # 4 channel groups per core

_CACHE = {}

MMCOLS = 512           # one PSUM bank of fp32 per matmul chunk
ZCHUNKS = (512, 256)        # PSUM-preload chunks: PE covers cols [0, 768)
ABCOLS = 2231               # AB-split region (ACT 2x + DVE 2x + Pool add)
DCOLS = 550                 # DVE-finished scratch cols; Pool finishes rest


def _build_program(zchunks=ZCHUNKS, xchunks=0, abcols=ABCOLS, dcols=DCOLS,
                   use_pool=True,
                   in_bufs=5, out_bufs=5, sc_bufs=3, tmp_bufs=2, preissue=1,
                   split_out=(), out_eng="pool", warmup=0):
    import concourse.bacc as bacc
    import concourse.mybir as mybir
    from concourse.tile import TileContext

    f32 = mybir.dt.float32
    bf16 = mybir.dt.bfloat16
    nc = bacc.Bacc("TRN2", target_bir_lowering=False, debug=False)

    xs = nc.dram_tensor("xs", [ROWS, L], f32, kind="ExternalInput").ap()
    # single packed param tensor -> single DMA -> single sync wait downstream.
    # layout per partition p: cols [g*4+j]=w[g*128+p, j] for g<4,j<4 (0..16),
    # col 16+g = bias[g*128+p], col 20+g*3+k = init_state[g*128+p, k]
    prm_d = nc.dram_tensor("prm", [P, 32], f32, kind="ExternalInput").ap()
    eye_d = nc.dram_tensor("eye", [P, P], f32, kind="ExternalInput").ap()
    out_d = nc.dram_tensor("out", [ROWS, LOUT], bf16, kind="ExternalOutput").ap()

    chunks = [(MMCOLS, False)] * xchunks + [(n, True) for n in zchunks]
    ncols = sum(n for n, _ in chunks)     # PE region [0, ncols)
    if not use_pool:
        abcols = 0
        dcols = LOUT - ncols
    a0 = ncols                            # AB-split region [a0, d0)
    d0 = a0 + abcols                      # scratch region [d0, LOUT)
    q0 = d0 + dcols                       # Pool-finished cols [q0, LOUT)
    scw = LOUT - d0                       # scratch width
    mcols = LOUT - q0

    # in_t layout: col 0 pad (16B align), state [1:4), x [4:4100),
    # zero tail [4100:4103) so the last taps can run off the end of x.
    XW = 1 + K + L + 4     # 4104 (16B-aligned row)

    with TileContext(nc) as tc:
        with (
            tc.tile_pool(name="consts", bufs=1) as cpool,
            tc.tile_pool(name="xin", bufs=in_bufs) as in_pool,
            tc.tile_pool(name="yout", bufs=out_bufs) as out_pool,
            tc.tile_pool(name="scr", bufs=sc_bufs) as sc_pool,
            tc.tile_pool(name="ab", bufs=tmp_bufs) as ab_pool,
            tc.tile_pool(name="tmp", bufs=tmp_bufs) as tmp_pool,
            tc.tile_pool(name="psum", bufs=8, space="PSUM") as ps_pool,
        ):
            # First in-DMAs go FIRST on the SP ring: the first big transfer
            # starts as early as the pipe allows and hides the small prm/eye
            # transfers' DGE latency behind it.
            pre = {}
            for t in range(preissue):
                in_t = in_pool.tile([P, XW], f32, name="in_t", tag="in_t")
                nc.sync.dma_start(out=in_t[:, 1 + K:1 + K + L],
                                  in_=xs[t * P:(t + 1) * P, :])
                pre[t] = in_t

            prm = cpool.tile([P, 32], f32)
            nc.sync.dma_start(out=prm, in_=prm_d)
            w_sb = prm[:, 0:G * KTAPS]
            b_sb = prm[:, 16:16 + G]
            s_sb = prm[:, 20:20 + G * K]

            # per-(group, tap) diagonal weight matrices for the PE path
            eye = cpool.tile([P, P], f32)
            nc.sync.dma_start(out=eye, in_=eye_d)

            if warmup:
                # Dummy matmuls while the first in-DMA streams: the PE
                # p-state needs >3us of continuous execution to reach full
                # clock, so tile 0's real matmuls start warm instead of at
                # the 2.8x-slower cold rate (which created a standing
                # backlog that stalled the in-DMA ring).
                wz = cpool.tile([P, MMCOLS], f32, tag="warmsrc")
                nc.vector.memset(wz, 0.0)
                wps = ps_pool.tile([P, MMCOLS], f32, tag="warmps")
                for _ in range(warmup):
                    nc.tensor.matmul(wps, eye, wz, start=True, stop=True)

            dg = {}
            for g in range(G):
                for j in range(KTAPS):
                    d = cpool.tile([P, P], f32, tag=f"diag{g}_{j}")
                    nc.vector.tensor_scalar_mul(
                        out=d, in0=eye,
                        scalar1=w_sb[:, g * KTAPS + j:g * KTAPS + j + 1])
                    dg[(g, j)] = d

            def stt(eng, out_t, in0, scal, in1):
                """out = in0*scal + in1 (fused MAC on eng)"""
                eng.scalar_tensor_tensor(
                    out=out_t, in0=in0, scalar=scal, in1=in1,
                    op0=mybir.AluOpType.mult, op1=mybir.AluOpType.add)

            for t in range(NTILES):
                g = t % G  # channel group (tile order: batch-major)
                rows = slice(t * P, (t + 1) * P)
                wj = [w_sb[:, g * KTAPS + j:g * KTAPS + j + 1]
                      for j in range(KTAPS)]

                if t in pre:
                    in_t = pre[t]
                else:
                    in_t = in_pool.tile([P, XW], f32, name="in_t", tag="in_t")
                    nc.sync.dma_start(out=in_t[:, 1 + K:1 + K + L],
                                      in_=xs[rows, :])
                nc.scalar.copy(in_t[:, 1:1 + K], s_sb[:, g * K:(g + 1) * K])
                nc.vector.memset(in_t[:, 1 + K + L:1 + K + L + K], 0.0)

                out_t = out_pool.tile([P, LOUT], bf16)

                # PE region: out[:, 0:ncols) accumulated in PSUM, ACT
                # evacuates + bias -> bf16. z-mode chunks pre-write tap0+bias
                # on ACT into the PSUM bank so only 3 matmuls accumulate on
                # top; x-mode chunks do all 4 taps as matmuls. All preloads
                # are issued before the matmul groups so the in-order ACT
                # ring stays ahead of the PE instead of serializing chunks.
                pss = []
                base = 1
                for n, zmode in chunks:
                    ps = ps_pool.tile([P, MMCOLS], f32)
                    pss.append(ps)
                    if zmode:
                        nc.scalar.activation(
                            ps[:, :n], in_t[:, base:base + n],
                            mybir.ActivationFunctionType.Identity,
                            bias=b_sb[:, g:g + 1], scale=wj[0])
                    base += n

                # AB-split region [a0, d0): the 4 taps split into two
                # independent fp32 partial sums A = w0*x0+b + w1*x1 and
                # B = w2*x2 + w3*x3 (ACT writes each base, DVE one fused MAC
                # each), and the otherwise-idle Pool adds A+B -> bf16 out
                # (single rounding). Spreads one column across 3 engines.
                if abcols:
                    sa = ab_pool.tile([P, abcols], f32, tag="sa")
                    sb = ab_pool.tile([P, abcols], f32, tag="sb")
                    nc.scalar.activation(
                        sa, in_t[:, 1 + a0:1 + a0 + abcols],
                        mybir.ActivationFunctionType.Identity,
                        bias=b_sb[:, g:g + 1], scale=wj[0])
                    nc.scalar.mul(
                        sb, in_t[:, 3 + a0:3 + a0 + abcols], wj[2])
                    stt(nc.vector, sa,
                        in_t[:, 2 + a0:2 + a0 + abcols], wj[1], sa)
                    stt(nc.vector, sb,
                        in_t[:, 4 + a0:4 + a0 + abcols], wj[3], sb)
                    nc.gpsimd.tensor_tensor(
                        out=out_t[:, a0:d0], in0=sa, in1=sb,
                        op=mybir.AluOpType.add)

                # Scratch-region tap0 on ACT.
                sc = sc_pool.tile([P, scw], f32)
                nc.scalar.activation(
                    sc, in_t[:, 1 + d0:1 + LOUT],
                    mybir.ActivationFunctionType.Identity,
                    bias=b_sb[:, g:g + 1], scale=wj[0])

                base = 1
                for ps, (n, zmode) in zip(pss, chunks):
                    for j in range(KTAPS):
                        if zmode and j == 0:
                            continue
                        nc.tensor.matmul(
                            ps[:, :n], dg[(g, j)],
                            in_t[:, base + j:base + j + n],
                            start=(j == 0), stop=(j == KTAPS - 1),
                            skip_group_check=zmode)
                    nc.scalar.activation(
                        out_t[:, base - 1:base - 1 + n], ps[:, :n],
                        mybir.ActivationFunctionType.Identity,
                        bias=(0.0 if zmode else b_sb[:, g:g + 1]),
                        scale=1.0)
                    base += n

                # Scratch region: DVE chains taps 1-2 into scratch over the
                # full width. Tap 3 is split: DVE finishes cols [d0, q0)
                # with a fused MAC writing bf16 out (single rounding); Pool
                # finishes [q0, LOUT) as tensor_scalar (tmp = x*w3) +
                # tensor_tensor add (out = tmp + scratch) since walrus has
                # no Pool scalar_tensor_tensor.
                for j in (1, 2):
                    stt(nc.vector, sc,
                        in_t[:, 1 + d0 + j:1 + LOUT + j], wj[j], sc)
                stt(nc.vector, out_t[:, d0:q0],
                    in_t[:, 1 + d0 + 3:1 + q0 + 3], wj[3],
                    sc[:, 0:q0 - d0])
                if use_pool:
                    tmp = tmp_pool.tile([P, mcols], f32)
                    nc.gpsimd.tensor_scalar(
                        out=tmp, in0=in_t[:, 1 + q0 + 3:1 + LOUT + 3],
                        scalar1=wj[3], scalar2=None,
                        op0=mybir.AluOpType.mult)
                    nc.gpsimd.tensor_tensor(
                        out=out_t[:, q0:], in0=tmp,
                        in1=sc[:, q0 - d0:], op=mybir.AluOpType.add)

                # SWDGE path: waits stall only the idle Pool sequencer;
                # the in-DMA HWDGE ring stays wait-free.
                oe = {"pool": nc.gpsimd, "act": nc.scalar, "sp": nc.sync,
                      "dve": nc.vector}[out_eng]
                if t in split_out:
                    # PE+DVE piece leaves as soon as their writes land; only
                    # the Pool-finished cols trail (shorter drain).
                    oe.dma_start(out=out_d[rows, :q0], in_=out_t[:, :q0])
                    oe.dma_start(out=out_d[rows, q0:], in_=out_t[:, q0:])
                else:
                    oe.dma_start(out=out_d[rows, :], in_=out_t)

    nc.compile()
    return nc


def kernel(x, weight, bias, init_state):
    from concourse.bass_utils import run_bass_kernel_spmd

    assert x.shape == (B, D, L) and x.dtype == np.float32
    wl = np.ascontiguousarray(weight[:, 0, :], dtype=np.float32)      # [D, 4]
    bias = np.ascontiguousarray(bias, dtype=np.float32)               # [D]
    st = np.ascontiguousarray(init_state, dtype=np.float32)           # [D, 3]

    if "nc" not in _CACHE:
        _CACHE["nc"] = _build_program()
    nc = _CACHE["nc"]

    in_maps = []
    for c in range(NCORES):
        lo, hi = c * DSH, (c + 1) * DSH
        xs = np.ascontiguousarray(x[:, lo:hi, :]).reshape(ROWS, L)
        wc = wl[lo:hi]                                                # [512, 4]
        prm = np.zeros((P, 32), np.float32)
        prm[:, 0:G * KTAPS] = (
            wc.reshape(G, P, KTAPS).transpose(1, 0, 2).reshape(P, G * KTAPS))
        prm[:, 16:16 + G] = bias[lo:hi].reshape(G, P).T
        prm[:, 20:20 + G * K] = (
            st[lo:hi].reshape(G, P, K).transpose(1, 0, 2).reshape(P, G * K))
        in_maps.append({"xs": xs, "prm": prm,
                        "eye": np.eye(P, dtype=np.float32)})

    res = run_bass_kernel_spmd(nc, in_maps, core_ids=list(range(NCORES)))
    shards = [r["out"].astype(np.float32).reshape(B, DSH, LOUT)
              for r in res.results]
    return np.ascontiguousarray(np.concatenate(shards, axis=1))


# revision 32
# speedup vs baseline: 1.1290x; 1.0401x over previous
"""Causal depthwise conv1d with learnable hidden-state prefix, on 8 TRN2 cores.

Reference computation (per batch b, channel d):
    xp = concat([init_state[d, :3], x[b, d, :]])          # [L+3] = [4099]
    out[b, d, t] = bias[d] + sum_{j=0..3} w[d, j] * xp[t+j]   for t in [0, 4099)
    (xp index beyond 4098 contributes 0)

Sharding: channel dim D=4096 split 8 ways (512 channels/core), zero
communication. Each core processes rows (b, d_local) = 4*512 = 2048 rows of
length 4096 -> 16 SBUF tiles of [128 rows, full row].

The output is stored to DRAM in bf16 (a single final rounding, ~2e-3 relative
error vs the 2e-2 gate; all accumulation stays fp32) and upcast to fp32 on the
host. That halves output DMA traffic, dropping the per-core DMA floor from
~187us (fp32 in+out) to ~140us (fp32 in + bf16 out).

At a 140us floor the old two-engine compute split (PE+DVE at ~153us each)
becomes the bottleneck, so the output columns are split two ways, both under
the DMA roofline (per-tile DMA budget is 8.75us: 5.83 in + 2.92 out):
  - PE "z-mode" chunks, cols [0, 1632): ACT pre-writes tap0 + bias into the
    PSUM bank, then only 3 diagonal-weight fp32 matmuls accumulate on top
    (start=False), ACT evacuates -> bf16. 3 matmuls/col instead of 4 keeps
    PE at ~8.2us/tile, inside the budget.
  - DVE region, cols [1632, 4099): ACT writes tap0+bias to an fp32 scratch,
    DVE chains taps 1-2 into scratch (scalar_tensor_tensor), tap 3 reads
    scratch and writes bf16 out directly (single rounding). ~8.2us/tile.
    The 3 zero-padded in_t cols let the last taps run off the end of x.
(Pool/GPSIMD only issues the SWDGE out-DMAs: walrus rejects
scalar_tensor_tensor on Pool, and with z-mode it isn't needed.)
"""

import numpy as np

B, D, L = 4, 4096, 4096
KTAPS = 4
K = KTAPS - 1          # 3: state length
LOUT = L + K           # 4099
NCORES = 8
DSH = D // NCORES      # 512 channels per core
ROWS = B * DSH         # 2048 rows per core
P = 128                # SBUF partitions
NTILES = ROWS // P     # 16
G = DSH // P           # 4 channel groups per core

_CACHE = {}

MMCOLS = 512           # one PSUM bank of fp32 per matmul chunk
ZCHUNKS = (512, 256)        # PSUM-preload chunks: PE covers cols [0, 768)
ABCOLS = 2231               # AB-split region (ACT 2x + DVE 2x + Pool add)
DCOLS = 550                 # DVE-finished scratch cols; Pool finishes rest


def _build_program(zchunks=ZCHUNKS, xchunks=0, abcols=ABCOLS, dcols=DCOLS,
                   use_pool=True,
                   in_bufs=5, out_bufs=5, sc_bufs=3, tmp_bufs=2, preissue=1,
                   split_out=(), out_eng="pool", warmup=0):
    import concourse.bacc as bacc
    import concourse.mybir as mybir
    from concourse.tile import TileContext

    f32 = mybir.dt.float32
    bf16 = mybir.dt.bfloat16
    nc = bacc.Bacc("TRN2", target_bir_lowering=False, debug=False)

    xs = nc.dram_tensor("xs", [ROWS, L], f32, kind="ExternalInput").ap()
    # single packed param tensor -> single DMA -> single sync wait downstream.
    # layout per partition p: cols [g*4+j]=w[g*128+p, j] for g<4,j<4 (0..16),
    # col 16+g = bias[g*128+p], col 20+g*3+k = init_state[g*128+p, k]
    prm_d = nc.dram_tensor("prm", [P, 32], f32, kind="ExternalInput").ap()
    eye_d = nc.dram_tensor("eye", [P, P], f32, kind="ExternalInput").ap()
    out_d = nc.dram_tensor("out", [ROWS, LOUT], bf16, kind="ExternalOutput").ap()

    chunks = [(MMCOLS, False)] * xchunks + [(n, True) for n in zchunks]
    ncols = sum(n for n, _ in chunks)     # PE region [0, ncols)
    if not use_pool:
        abcols = 0
        dcols = LOUT - ncols
    a0 = ncols                            # AB-split region [a0, d0)
    d0 = a0 + abcols                      # scratch region [d0, LOUT)
    q0 = d0 + dcols                       # Pool-finished cols [q0, LOUT)
    scw = LOUT - d0                       # scratch width
    mcols = LOUT - q0

    # in_t layout: col 0 pad (16B align), state [1:4), x [4:4100),
    # zero tail [4100:4103) so the last taps can run off the end of x.
    XW = 1 + K + L + 4     # 4104 (16B-aligned row)

    with TileContext(nc) as tc:
        with (
            tc.tile_pool(name="consts", bufs=1) as cpool,
            tc.tile_pool(name="xin", bufs=in_bufs) as in_pool,
            tc.tile_pool(name="yout", bufs=out_bufs) as out_pool,
            tc.tile_pool(name="scr", bufs=sc_bufs) as sc_pool,
            tc.tile_pool(name="ab", bufs=tmp_bufs) as ab_pool,
            tc.tile_pool(name="tmp", bufs=tmp_bufs) as tmp_pool,
            tc.tile_pool(name="psum", bufs=8, space="PSUM") as ps_pool,
        ):
            # First in-DMAs go FIRST on the SP ring: the first big transfer
            # starts as early as the pipe allows and hides the small prm/eye
            # transfers' DGE latency behind it.
            pre = {}
            for t in range(preissue):
                in_t = in_pool.tile([P, XW], f32, name="in_t", tag="in_t")
                nc.sync.dma_start(out=in_t[:, 1 + K:1 + K + L],
                                  in_=xs[t * P:(t + 1) * P, :])
                pre[t] = in_t

            prm = cpool.tile([P, 32], f32)
            nc.sync.dma_start(out=prm, in_=prm_d)
            w_sb = prm[:, 0:G * KTAPS]
            b_sb = prm[:, 16:16 + G]
            s_sb = prm[:, 20:20 + G * K]

            # per-(group, tap) diagonal weight matrices for the PE path
            eye = cpool.tile([P, P], f32)
            nc.sync.dma_start(out=eye, in_=eye_d)

            if warmup:
                # Dummy matmuls while the first in-DMA streams: the PE
                # p-state needs >3us of continuous execution to reach full
                # clock, so tile 0's real matmuls start warm instead of at
                # the 2.8x-slower cold rate (which created a standing
                # backlog that stalled the in-DMA ring).
                wz = cpool.tile([P, MMCOLS], f32, tag="warmsrc")
                nc.vector.memset(wz, 0.0)
                for _ in range(warmup):
                    ps = ps_pool.tile([P, MMCOLS], f32, name="ps")
                    nc.tensor.matmul(ps, eye, wz, start=True, stop=True)

            dg = {}
            for g in range(G):
                for j in range(KTAPS):
                    d = cpool.tile([P, P], f32, tag=f"diag{g}_{j}")
                    nc.vector.tensor_scalar_mul(
                        out=d, in0=eye,
                        scalar1=w_sb[:, g * KTAPS + j:g * KTAPS + j + 1])
                    dg[(g, j)] = d

            def stt(eng, out_t, in0, scal, in1):
                """out = in0*scal + in1 (fused MAC on eng)"""
                eng.scalar_tensor_tensor(
                    out=out_t, in0=in0, scalar=scal, in1=in1,
                    op0=mybir.AluOpType.mult, op1=mybir.AluOpType.add)

            for t in range(NTILES):
                g = t % G  # channel group (tile order: batch-major)
                rows = slice(t * P, (t + 1) * P)
                wj = [w_sb[:, g * KTAPS + j:g * KTAPS + j + 1]
                      for j in range(KTAPS)]

                if t in pre:
                    in_t = pre[t]
                else:
                    in_t = in_pool.tile([P, XW], f32, name="in_t", tag="in_t")
                    nc.sync.dma_start(out=in_t[:, 1 + K:1 + K + L],
                                      in_=xs[rows, :])
                nc.scalar.copy(in_t[:, 1:1 + K], s_sb[:, g * K:(g + 1) * K])
                nc.vector.memset(in_t[:, 1 + K + L:1 + K + L + K], 0.0)

                out_t = out_pool.tile([P, LOUT], bf16)

                # PE region: out[:, 0:ncols) accumulated in PSUM, ACT
                # evacuates + bias -> bf16. z-mode chunks pre-write tap0+bias
                # on ACT into the PSUM bank so only 3 matmuls accumulate on
                # top; x-mode chunks do all 4 taps as matmuls. All preloads
                # are issued before the matmul groups so the in-order ACT
                # ring stays ahead of the PE instead of serializing chunks.
                pss = []
                base = 1
                for n, zmode in chunks:
                    ps = ps_pool.tile([P, MMCOLS], f32)
                    pss.append(ps)
                    if zmode:
                        nc.scalar.activation(
                            ps[:, :n], in_t[:, base:base + n],
                            mybir.ActivationFunctionType.Identity,
                            bias=b_sb[:, g:g + 1], scale=wj[0])
                    base += n

                # AB-split region [a0, d0): the 4 taps split into two
                # independent fp32 partial sums A = w0*x0+b + w1*x1 and
                # B = w2*x2 + w3*x3 (ACT writes each base, DVE one fused MAC
                # each), and the otherwise-idle Pool adds A+B -> bf16 out
                # (single rounding). Spreads one column across 3 engines.
                if abcols:
                    sa = ab_pool.tile([P, abcols], f32, tag="sa")
                    sb = ab_pool.tile([P, abcols], f32, tag="sb")
                    nc.scalar.activation(
                        sa, in_t[:, 1 + a0:1 + a0 + abcols],
                        mybir.ActivationFunctionType.Identity,
                        bias=b_sb[:, g:g + 1], scale=wj[0])
                    nc.scalar.mul(
                        sb, in_t[:, 3 + a0:3 + a0 + abcols], wj[2])
                    stt(nc.vector, sa,
                        in_t[:, 2 + a0:2 + a0 + abcols], wj[1], sa)
                    stt(nc.vector, sb,
                        in_t[:, 4 + a0:4 + a0 + abcols], wj[3], sb)
                    nc.gpsimd.tensor_tensor(
                        out=out_t[:, a0:d0], in0=sa, in1=sb,
                        op=mybir.AluOpType.add)

                # Scratch-region tap0 on ACT.
                sc = sc_pool.tile([P, scw], f32)
                nc.scalar.activation(
                    sc, in_t[:, 1 + d0:1 + LOUT],
                    mybir.ActivationFunctionType.Identity,
                    bias=b_sb[:, g:g + 1], scale=wj[0])

                base = 1
                for ps, (n, zmode) in zip(pss, chunks):
                    for j in range(KTAPS):
                        if zmode and j == 0:
                            continue
                        nc.tensor.matmul(
                            ps[:, :n], dg[(g, j)],
                            in_t[:, base + j:base + j + n],
                            start=(j == 0), stop=(j == KTAPS - 1),
                            skip_group_check=zmode)
                    nc.scalar.activation(
                        out_t[:, base - 1:base - 1 + n], ps[:, :n],
                        mybir.ActivationFunctionType.Identity,
                        bias=(0.0 if zmode else b_sb[:, g:g + 1]),
                        scale=1.0)
                    base += n

                # Scratch region: DVE chains taps 1-2 into scratch over the
                # full width. Tap 3 is split: DVE finishes cols [d0, q0)
                # with a fused MAC writing bf16 out (single rounding); Pool
                # finishes [q0, LOUT) as tensor_scalar (tmp = x*w3) +
                # tensor_tensor add (out = tmp + scratch) since walrus has
                # no Pool scalar_tensor_tensor.
                for j in (1, 2):
                    stt(nc.vector, sc,
                        in_t[:, 1 + d0 + j:1 + LOUT + j], wj[j], sc)
                stt(nc.vector, out_t[:, d0:q0],
                    in_t[:, 1 + d0 + 3:1 + q0 + 3], wj[3],
                    sc[:, 0:q0 - d0])
                if use_pool:
                    tmp = tmp_pool.tile([P, mcols], f32)
                    nc.gpsimd.tensor_scalar(
                        out=tmp, in0=in_t[:, 1 + q0 + 3:1 + LOUT + 3],
                        scalar1=wj[3], scalar2=None,
                        op0=mybir.AluOpType.mult)
                    nc.gpsimd.tensor_tensor(
                        out=out_t[:, q0:], in0=tmp,
                        in1=sc[:, q0 - d0:], op=mybir.AluOpType.add)

                # SWDGE path: waits stall only the idle Pool sequencer;
                # the in-DMA HWDGE ring stays wait-free.
                oe = {"pool": nc.gpsimd, "act": nc.scalar, "sp": nc.sync,
                      "dve": nc.vector}[out_eng]
                if t in split_out:
                    # PE+DVE piece leaves as soon as their writes land; only
                    # the Pool-finished cols trail (shorter drain).
                    oe.dma_start(out=out_d[rows, :q0], in_=out_t[:, :q0])
                    oe.dma_start(out=out_d[rows, q0:], in_=out_t[:, q0:])
                else:
                    oe.dma_start(out=out_d[rows, :], in_=out_t)

    nc.compile()
    return nc


def kernel(x, weight, bias, init_state):
    from concourse.bass_utils import run_bass_kernel_spmd

    assert x.shape == (B, D, L) and x.dtype == np.float32
    wl = np.ascontiguousarray(weight[:, 0, :], dtype=np.float32)      # [D, 4]
    bias = np.ascontiguousarray(bias, dtype=np.float32)               # [D]
    st = np.ascontiguousarray(init_state, dtype=np.float32)           # [D, 3]

    if "nc" not in _CACHE:
        _CACHE["nc"] = _build_program()
    nc = _CACHE["nc"]

    in_maps = []
    for c in range(NCORES):
        lo, hi = c * DSH, (c + 1) * DSH
        xs = np.ascontiguousarray(x[:, lo:hi, :]).reshape(ROWS, L)
        wc = wl[lo:hi]                                                # [512, 4]
        prm = np.zeros((P, 32), np.float32)
        prm[:, 0:G * KTAPS] = (
            wc.reshape(G, P, KTAPS).transpose(1, 0, 2).reshape(P, G * KTAPS))
        prm[:, 16:16 + G] = bias[lo:hi].reshape(G, P).T
        prm[:, 20:20 + G * K] = (
            st[lo:hi].reshape(G, P, K).transpose(1, 0, 2).reshape(P, G * K))
        in_maps.append({"xs": xs, "prm": prm,
                        "eye": np.eye(P, dtype=np.float32)})

    res = run_bass_kernel_spmd(nc, in_maps, core_ids=list(range(NCORES)))
    shards = [r["out"].astype(np.float32).reshape(B, DSH, LOUT)
              for r in res.results]
    return np.ascontiguousarray(np.concatenate(shards, axis=1))


# revision 37
# speedup vs baseline: 1.1693x; 1.0357x over previous
"""Causal depthwise conv1d with learnable hidden-state prefix, on 8 TRN2 cores.

Reference computation (per batch b, channel d):
    xp = concat([init_state[d, :3], x[b, d, :]])          # [L+3] = [4099]
    out[b, d, t] = bias[d] + sum_{j=0..3} w[d, j] * xp[t+j]   for t in [0, 4099)
    (xp index beyond 4098 contributes 0)

Sharding: channel dim D=4096 split 8 ways (512 channels/core), zero
communication. Each core processes rows (b, d_local) = 4*512 = 2048 rows of
length 4096 -> 16 SBUF tiles of [128 rows, full row].

The output is stored to DRAM in bf16 (a single final rounding, ~2e-3 relative
error vs the 2e-2 gate; all accumulation stays fp32) and upcast to fp32 on the
host. That halves output DMA traffic, dropping the per-core DMA floor from
~187us (fp32 in+out) to ~140us (fp32 in + bf16 out).

At a 140us floor the old two-engine compute split (PE+DVE at ~153us each)
becomes the bottleneck, so the output columns are split two ways, both under
the DMA roofline (per-tile DMA budget is 8.75us: 5.83 in + 2.92 out):
  - PE "z-mode" chunks, cols [0, 1632): ACT pre-writes tap0 + bias into the
    PSUM bank, then only 3 diagonal-weight fp32 matmuls accumulate on top
    (start=False), ACT evacuates -> bf16. 3 matmuls/col instead of 4 keeps
    PE at ~8.2us/tile, inside the budget.
  - DVE region, cols [1632, 4099): ACT writes tap0+bias to an fp32 scratch,
    DVE chains taps 1-2 into scratch (scalar_tensor_tensor), tap 3 reads
    scratch and writes bf16 out directly (single rounding). ~8.2us/tile.
    The 3 zero-padded in_t cols let the last taps run off the end of x.
(Pool/GPSIMD only issues the SWDGE out-DMAs: walrus rejects
scalar_tensor_tensor on Pool, and with z-mode it isn't needed.)
"""

import numpy as np

B, D, L = 4, 4096, 4096
KTAPS = 4
K = KTAPS - 1          # 3: state length
LOUT = L + K           # 4099
NCORES = 8
DSH = D // NCORES      # 512 channels per core
ROWS = B * DSH         # 2048 rows per core
P = 128                # SBUF partitions
NTILES = ROWS // P     # 16
G = DSH // P           # 4 channel groups per core

_CACHE = {}

MMCOLS = 512           # one PSUM bank of fp32 per matmul chunk
ZCHUNKS = (512, 256)        # PSUM-preload chunks: PE covers cols [0, 768)
ABCOLS = 2231               # AB-split region (ACT 2x + DVE 2x + Pool add)
DCOLS = 550                 # DVE-finished scratch cols; Pool finishes rest


def _build_program(zchunks=ZCHUNKS, xchunks=0, abcols=ABCOLS, dcols=DCOLS,
                   use_pool=True,
                   in_bufs=5, out_bufs=5, sc_bufs=3, tmp_bufs=2, preissue=1,
                   split_out=(), out_eng="pool", warmup=0):
    import concourse.bacc as bacc
    import concourse.mybir as mybir
    from concourse.tile import TileContext

    f32 = mybir.dt.float32
    bf16 = mybir.dt.bfloat16
    nc = bacc.Bacc("TRN2", target_bir_lowering=False, debug=False)

    xs = nc.dram_tensor("xs", [ROWS, L], f32, kind="ExternalInput").ap()
    # single packed param tensor -> single DMA -> single sync wait downstream.
    # layout per partition p: cols [g*4+j]=w[g*128+p, j] for g<4,j<4 (0..16),
    # col 16+g = bias[g*128+p], col 20+g*3+k = init_state[g*128+p, k]
    # prm cols [0,32); eye cols [32,160) -- one DMA, one sync downstream
    prm_d = nc.dram_tensor("prm", [P, 32 + P], f32, kind="ExternalInput").ap()
    out_d = nc.dram_tensor("out", [ROWS, LOUT], bf16, kind="ExternalOutput").ap()

    chunks = [(MMCOLS, False)] * xchunks + [(n, True) for n in zchunks]
    ncols = sum(n for n, _ in chunks)     # PE region [0, ncols)
    if not use_pool:
        abcols = 0
        dcols = LOUT - ncols
    a0 = ncols                            # AB-split region [a0, d0)
    d0 = a0 + abcols                      # scratch region [d0, LOUT)
    q0 = d0 + dcols                       # Pool-finished cols [q0, LOUT)
    scw = LOUT - d0                       # scratch width
    mcols = LOUT - q0

    # in_t layout: col 0 pad (16B align), state [1:4), x [4:4100),
    # zero tail [4100:4103) so the last taps can run off the end of x.
    XW = 1 + K + L + 4     # 4104 (16B-aligned row)

    with TileContext(nc) as tc:
        with (
            tc.tile_pool(name="consts", bufs=1) as cpool,
            tc.tile_pool(name="xin", bufs=in_bufs) as in_pool,
            tc.tile_pool(name="yout", bufs=out_bufs) as out_pool,
            tc.tile_pool(name="scr", bufs=sc_bufs) as sc_pool,
            tc.tile_pool(name="ab", bufs=tmp_bufs) as ab_pool,
            tc.tile_pool(name="tmp", bufs=tmp_bufs) as tmp_pool,
            tc.tile_pool(name="psum", bufs=8, space="PSUM") as ps_pool,
        ):
            # First in-DMAs go FIRST on the SP ring: the first big transfer
            # starts as early as the pipe allows and hides the small prm/eye
            # transfers' DGE latency behind it.
            pre = {}
            for t in range(preissue):
                in_t = in_pool.tile([P, XW], f32, name="in_t", tag="in_t")
                nc.sync.dma_start(out=in_t[:, 1 + K:1 + K + L],
                                  in_=xs[t * P:(t + 1) * P, :])
                pre[t] = in_t

            prm = cpool.tile([P, 32 + P], f32)
            nc.sync.dma_start(out=prm, in_=prm_d)
            w_sb = prm[:, 0:G * KTAPS]
            b_sb = prm[:, 16:16 + G]
            s_sb = prm[:, 20:20 + G * K]
            eye = prm[:, 32:32 + P]

            if warmup:
                # Dummy matmuls while the first in-DMA streams: the PE
                # p-state needs >3us of continuous execution to reach full
                # clock, so tile 0's real matmuls start warm instead of at
                # the 2.8x-slower cold rate (which created a standing
                # backlog that stalled the in-DMA ring).
                wz = cpool.tile([P, MMCOLS], f32, tag="warmsrc")
                nc.vector.memset(wz, 0.0)
                for _ in range(warmup):
                    ps = ps_pool.tile([P, MMCOLS], f32, name="ps")
                    nc.tensor.matmul(ps, eye, wz, start=True, stop=True)

            dg = {}
            for g in range(G):
                for j in range(KTAPS):
                    d = cpool.tile([P, P], f32, tag=f"diag{g}_{j}")
                    nc.vector.tensor_scalar_mul(
                        out=d, in0=eye,
                        scalar1=w_sb[:, g * KTAPS + j:g * KTAPS + j + 1])
                    dg[(g, j)] = d

            def stt(eng, out_t, in0, scal, in1):
                """out = in0*scal + in1 (fused MAC on eng)"""
                eng.scalar_tensor_tensor(
                    out=out_t, in0=in0, scalar=scal, in1=in1,
                    op0=mybir.AluOpType.mult, op1=mybir.AluOpType.add)

            for t in range(NTILES):
                g = t % G  # channel group (tile order: batch-major)
                rows = slice(t * P, (t + 1) * P)
                wj = [w_sb[:, g * KTAPS + j:g * KTAPS + j + 1]
                      for j in range(KTAPS)]

                if t in pre:
                    in_t = pre[t]
                else:
                    in_t = in_pool.tile([P, XW], f32, name="in_t", tag="in_t")
                    nc.sync.dma_start(out=in_t[:, 1 + K:1 + K + L],
                                      in_=xs[rows, :])
                nc.scalar.copy(in_t[:, 1:1 + K], s_sb[:, g * K:(g + 1) * K])
                if t < in_bufs + preissue:
                    # zero the 3-col tap tail once per physical buffer; the
                    # in-DMA never overwrites these columns, so later tiles
                    # reuse the zeros.
                    nc.vector.memset(in_t[:, 1 + K + L:1 + K + L + K], 0.0)

                out_t = out_pool.tile([P, LOUT], bf16)

                # PE region: out[:, 0:ncols) accumulated in PSUM, ACT
                # evacuates + bias -> bf16. z-mode chunks pre-write tap0+bias
                # on ACT into the PSUM bank so only 3 matmuls accumulate on
                # top; x-mode chunks do all 4 taps as matmuls. All preloads
                # are issued before the matmul groups so the in-order ACT
                # ring stays ahead of the PE instead of serializing chunks.
                pss = []
                base = 1
                for n, zmode in chunks:
                    ps = ps_pool.tile([P, MMCOLS], f32)
                    pss.append(ps)
                    if zmode:
                        nc.scalar.activation(
                            ps[:, :n], in_t[:, base:base + n],
                            mybir.ActivationFunctionType.Identity,
                            bias=b_sb[:, g:g + 1], scale=wj[0])
                    base += n

                # AB-split region [a0, d0): the 4 taps split into two
                # independent fp32 partial sums A = w0*x0+b + w1*x1 and
                # B = w2*x2 + w3*x3 (ACT writes each base, DVE one fused MAC
                # each), and the otherwise-idle Pool adds A+B -> bf16 out
                # (single rounding). Spreads one column across 3 engines.
                if abcols:
                    sa = ab_pool.tile([P, abcols], f32, tag="sa")
                    sb = ab_pool.tile([P, abcols], f32, tag="sb")
                    nc.scalar.activation(
                        sa, in_t[:, 1 + a0:1 + a0 + abcols],
                        mybir.ActivationFunctionType.Identity,
                        bias=b_sb[:, g:g + 1], scale=wj[0])
                    nc.scalar.mul(
                        sb, in_t[:, 3 + a0:3 + a0 + abcols], wj[2])
                    stt(nc.vector, sa,
                        in_t[:, 2 + a0:2 + a0 + abcols], wj[1], sa)
                    stt(nc.vector, sb,
                        in_t[:, 4 + a0:4 + a0 + abcols], wj[3], sb)
                    nc.gpsimd.tensor_tensor(
                        out=out_t[:, a0:d0], in0=sa, in1=sb,
                        op=mybir.AluOpType.add)

                # Scratch-region tap0 on ACT.
                sc = sc_pool.tile([P, scw], f32)
                nc.scalar.activation(
                    sc, in_t[:, 1 + d0:1 + LOUT],
                    mybir.ActivationFunctionType.Identity,
                    bias=b_sb[:, g:g + 1], scale=wj[0])

                base = 1
                for ps, (n, zmode) in zip(pss, chunks):
                    for j in range(KTAPS):
                        if zmode and j == 0:
                            continue
                        nc.tensor.matmul(
                            ps[:, :n], dg[(g, j)],
                            in_t[:, base + j:base + j + n],
                            start=(j == 0), stop=(j == KTAPS - 1),
                            skip_group_check=zmode)
                    nc.scalar.activation(
                        out_t[:, base - 1:base - 1 + n], ps[:, :n],
                        mybir.ActivationFunctionType.Identity,
                        bias=(0.0 if zmode else b_sb[:, g:g + 1]),
                        scale=1.0)
                    base += n

                # Scratch region: DVE chains taps 1-2 into scratch over the
                # full width. Tap 3 is split: DVE finishes cols [d0, q0)
                # with a fused MAC writing bf16 out (single rounding); Pool
                # finishes [q0, LOUT) as tensor_scalar (tmp = x*w3) +
                # tensor_tensor add (out = tmp + scratch) since walrus has
                # no Pool scalar_tensor_tensor.
                for j in (1, 2):
                    stt(nc.vector, sc,
                        in_t[:, 1 + d0 + j:1 + LOUT + j], wj[j], sc)
                stt(nc.vector, out_t[:, d0:q0],
                    in_t[:, 1 + d0 + 3:1 + q0 + 3], wj[3],
                    sc[:, 0:q0 - d0])
                if use_pool:
                    tmp = tmp_pool.tile([P, mcols], f32)
                    nc.gpsimd.tensor_scalar(
                        out=tmp, in0=in_t[:, 1 + q0 + 3:1 + LOUT + 3],
                        scalar1=wj[3], scalar2=None,
                        op0=mybir.AluOpType.mult)
                    nc.gpsimd.tensor_tensor(
                        out=out_t[:, q0:], in0=tmp,
                        in1=sc[:, q0 - d0:], op=mybir.AluOpType.add)

                # SWDGE path: waits stall only the idle Pool sequencer;
                # the in-DMA HWDGE ring stays wait-free.
                oe = {"pool": nc.gpsimd, "act": nc.scalar, "sp": nc.sync,
                      "dve": nc.vector}[out_eng]
                if t in split_out:
                    # PE+DVE piece leaves as soon as their writes land; only
                    # the Pool-finished cols trail (shorter drain).
                    oe.dma_start(out=out_d[rows, :q0], in_=out_t[:, :q0])
                    oe.dma_start(out=out_d[rows, q0:], in_=out_t[:, q0:])
                else:
                    oe.dma_start(out=out_d[rows, :], in_=out_t)

    nc.compile()
    return nc


def kernel(x, weight, bias, init_state):
    from concourse.bass_utils import run_bass_kernel_spmd

    assert x.shape == (B, D, L) and x.dtype == np.float32
    wl = np.ascontiguousarray(weight[:, 0, :], dtype=np.float32)      # [D, 4]
    bias = np.ascontiguousarray(bias, dtype=np.float32)               # [D]
    st = np.ascontiguousarray(init_state, dtype=np.float32)           # [D, 3]

    if "nc" not in _CACHE:
        _CACHE["nc"] = _build_program()
    nc = _CACHE["nc"]

    in_maps = []
    for c in range(NCORES):
        lo, hi = c * DSH, (c + 1) * DSH
        xs = np.ascontiguousarray(x[:, lo:hi, :]).reshape(ROWS, L)
        wc = wl[lo:hi]                                                # [512, 4]
        prm = np.zeros((P, 32 + P), np.float32)
        prm[:, 32:32 + P] = np.eye(P, dtype=np.float32)
        prm[:, 0:G * KTAPS] = (
            wc.reshape(G, P, KTAPS).transpose(1, 0, 2).reshape(P, G * KTAPS))
        prm[:, 16:16 + G] = bias[lo:hi].reshape(G, P).T
        prm[:, 20:20 + G * K] = (
            st[lo:hi].reshape(G, P, K).transpose(1, 0, 2).reshape(P, G * K))
        in_maps.append({"xs": xs, "prm": prm})

    res = run_bass_kernel_spmd(nc, in_maps, core_ids=list(range(NCORES)))
    shards = [r["out"].astype(np.float32).reshape(B, DSH, LOUT)
              for r in res.results]
    return np.ascontiguousarray(np.concatenate(shards, axis=1))
